# revision 20
# baseline (speedup 1.0000x reference)
"""TRN2 Bass kernel for nn_CommLayer (gnn message passing).

Math: x [B=65536, 512] viewed as [B, 8 agents, 64]; per agent a:
    y_a = tanh(x_a @ Wh.T + (sum_{a'!=a} x_{a'}) @ Wc.T / 7)
Rewritten with S = sum_a x_a:
    y_a = tanh(x_a @ U + S @ V),  U = Wh.T - Wc.T/7,  V = Wc.T/7

Sharding: data-parallel over batch across 8 NeuronCores (8192 rows each).

The kernel runs entirely in the transposed world so the PE never
transposes: the host ships Z = x^T as fp16 tiles [128, 4*512]
(partition = feature%128, free = chunk-major 512-row groups) plus the
per-tile agent sums S^T [64, 8192] (one 1 MiB load), and gets back Y^T
tiles in the z layout.  Per 512-row tile:
  - 4 matmuls, stationary blockdiag(U,U) [128,128]: y1^T chunks into two
    2-bank PSUM pair tiles [128, 1024]
  - 4 matmuls, stationary tile(V,(1,2)) [64,128], moving the S^T slice:
    accumulate the S@V term into the same pairs
  - 2 tanh on ScalarE (one per pair), PSUM f32 -> fp16 output tile
fp16 halves HBM traffic vs f32 (roofline ~358 GB/s/core) and runs the
PE at 1 cycle/row; PSUM stays f32 (measured end-to-end max abs err
~6e-3 vs the f32 reference, gate 2e-2).

Scheduling notes (all measured on HW traces):
  - PSUM pair pool bufs=4 = all 8 banks -> 2 tiles of PE-ahead slack, so
    the PE issues long gap-free matmul bursts; that keeps the HAM clock
    gate at 8/8 (a tighter 2-quad layout left the PE at 1.2 GHz for the
    whole kernel).
  - A dummy activation right after the weight DMA pays the one-time
    ~2.7us tanh table load during the first z load instead of on the
    critical path.
  - Input DMAs ride the sync HWDGE ring with a 6-tile prefetch window;
    tile 0 is loaded as four chunk DMAs so the first matmul starts
    ~2us earlier.  Output DMAs ride the gpsimd SWDGE ring; the last
    tile stores per-pair so the tail drains as tanhs retire.
"""
import sys

sys.path.insert(0, "/opt/trn_rl_repo")

import numpy as np

BATCH = 65536
D = 512
NAGENT = 8
DA = 64
NORM = NAGENT - 1
NCORES = 8
SHARD = BATCH // NCORES   # 8192 rows per core
TROWS = 512               # rows per tile
NT = SHARD // TROWS       # 16 tiles
NCHUNK = D // 128         # 4 feature chunks
FREE = NCHUNK * TROWS     # 2048 free elems per tile
HALF = FREE // 2          # 1024 (one PSUM pair)

_CACHE: dict = {}


def _build_nc():
    import concourse.mybir as mybir
    import concourse.tile as tile
    from concourse import bacc

    nc = bacc.Bacc("TRN2", target_bir_lowering=False, debug=False)

    f16 = mybir.dt.float16
    f32 = mybir.dt.float32
    Tanh = mybir.ActivationFunctionType.Tanh

    z_d = nc.dram_tensor("z", [NT, 128, FREE], f16, kind="ExternalInput")
    # S^T duplicated to 128 partitions so the p2 stationary is K=128:
    # a K=64 stationary leaves half the PE array idle and the HAM clock
    # gate never registers "busy" -> the whole kernel runs at 1.2 GHz.
    s_d = nc.dram_tensor("s", [128, NT * TROWS], f16, kind="ExternalInput")
    w_d = nc.dram_tensor("w", [128, 256], f16, kind="ExternalInput")
    y_d = nc.dram_tensor("y", [NT, 128, FREE], f16, kind="ExternalOutput")

    PRE = 8  # load prefetch depth (tiles ahead of compute)

    with tile.TileContext(nc) as tc:
        with (
            tc.tile_pool(name="const", bufs=1) as const,
            tc.tile_pool(name="zg", bufs=PRE + 2) as zgp,
            tc.tile_pool(name="yg", bufs=5) as ygp,
            tc.tile_pool(name="psp", bufs=4, space="PSUM") as pspp,
        ):
            w2 = const.tile([128, 256], f16)
            nc.sync.dma_start(w2[:], w_d[:])
            wd2 = w2[:, 0:128]
            wv = w2[:, 128:256]

            z_tiles = {}
            st = const.tile([128, NT * TROWS], f16)

            # a single HWDGE ring sustains only ~220 GB/s; alternate the
            # z loads (and the per-tile 128 KiB s slices) across the sync
            # and scalar rings so both run.  Tile 0 spreads its chunks
            # over three queues to beat the cold-HBM ramp.
            def load(t, split=False):
                zg = zgp.tile([128, FREE], f16, tag="zg", name=f"z{t}")
                eng = nc.sync if t % 2 == 0 else nc.scalar
                if split:
                    engs = [nc.sync, nc.scalar, nc.gpsimd, nc.sync]
                    for c in range(NCHUNK):
                        engs[c].dma_start(
                            zg[:, c * TROWS:(c + 1) * TROWS],
                            z_d[t, :, c * TROWS:(c + 1) * TROWS],
                        )
                else:
                    eng.dma_start(zg[:], z_d[t, :, :])
                eng.dma_start(
                    st[:, t * TROWS:(t + 1) * TROWS],
                    s_d[:, t * TROWS:(t + 1) * TROWS],
                )
                z_tiles[t] = zg

            load(0, split=True)

            # one-time tanh table load, paid under the z0/s0 DMAs
            warm = const.tile([128, 8], f16)
            nc.scalar.activation(warm[:], w2[:, 0:8], Tanh)

            for t in range(1, min(PRE, NT)):
                load(t)

            for t in range(NT):
                if t + PRE < NT:
                    load(t + PRE)
                zg = z_tiles.pop(t)
                pa = pspp.tile([128, HALF], f32, tag="psp", name=f"pa{t}")
                pb = pspp.tile([128, HALF], f32, tag="psp", name=f"pb{t}")
                halves = [pa, pa, pb, pb]
                for c in range(NCHUNK):
                    nc.tensor.matmul(
                        halves[c][:, (c % 2) * TROWS:(c % 2 + 1) * TROWS],
                        wd2, zg[:, c * TROWS:(c + 1) * TROWS],
                        start=True, stop=False,
                    )
                sm = st[:, t * TROWS:(t + 1) * TROWS]
                for c in range(NCHUNK):
                    nc.tensor.matmul(
                        halves[c][:, (c % 2) * TROWS:(c % 2 + 1) * TROWS],
                        wv, sm,
                        start=False, stop=True,
                    )
                yg = ygp.tile([128, FREE], f16, tag="yg", name=f"y{t}")
                nc.scalar.activation(yg[:, 0:HALF], pa[:], Tanh)
                if t == NT - 1:
                    nc.gpsimd.dma_start(y_d[t, :, 0:HALF], yg[:, 0:HALF])
                    nc.scalar.activation(yg[:, HALF:FREE], pb[:], Tanh)
                    nc.gpsimd.dma_start(y_d[t, :, HALF:FREE], yg[:, HALF:FREE])
                else:
                    nc.scalar.activation(yg[:, HALF:FREE], pb[:], Tanh)
                    nc.gpsimd.dma_start(y_d[t, :, :], yg[:])

    nc.compile()
    return nc


def _get_nc():
    if "nc" not in _CACHE:
        _CACHE["nc"] = _build_nc()
    return _CACHE["nc"]


def _build_weights(hw: np.ndarray, cw: np.ndarray):
    """Stationary operands [128, 256] fp16: wd2 | wv-padded (out = st^T @ mv)."""
    U = (hw.T - cw.T / np.float32(NORM)).astype(np.float32)
    V = (cw.T / np.float32(NORM)).astype(np.float32)
    wd2 = np.kron(np.eye(2, dtype=np.float32), U)           # [128, 128]
    wv = np.tile(V, (2, 2)) / 2.0                           # [128, 128], K=128
    return np.concatenate([wd2, wv], axis=1).astype(np.float16)


def _shard_inputs(x: np.ndarray, hw: np.ndarray, cw: np.ndarray):
    w = _build_weights(hw, cw)
    # z[core][t, p, c*512+r] = x[core*SHARD + t*TROWS + r, c*128 + p]
    xt = (
        x.reshape(NCORES, NT, TROWS, NCHUNK, 128)
        .transpose(0, 1, 4, 3, 2)
        .reshape(NCORES, NT, 128, FREE)
        .astype(np.float16)
    )
    # s[core][d%64, t*512+r] = sum_a x[core*SHARD + t*TROWS + r, a*64 + d%64]
    # duplicated along partitions to keep the p2 stationary at K=128
    s = (
        x.reshape(NCORES, SHARD, NAGENT, DA)
        .sum(axis=2)
        .transpose(0, 2, 1)
        .astype(np.float16)
    )
    s = np.concatenate([s, s], axis=1)                      # [NCORES, 128, SHARD]
    return [
        {"z": np.ascontiguousarray(xt[i]), "s": np.ascontiguousarray(s[i]),
         "w": w}
        for i in range(NCORES)
    ]


def _gather_output(results) -> np.ndarray:
    # y[core*SHARD + t*TROWS + r, c*128 + p] = yt[core][t, p, c*512+r]
    yt = np.stack([r["y"] for r in results])
    return (
        yt.reshape(NCORES, NT, 128, NCHUNK, TROWS)
        .transpose(0, 1, 4, 3, 2)
        .reshape(BATCH, D)
        .astype(np.float32)
    )


def kernel(**inputs) -> np.ndarray:
    from concourse.bass_utils import run_bass_kernel_spmd

    x = np.ascontiguousarray(np.asarray(inputs["x"], dtype=np.float32))
    hw = np.asarray(inputs["hidden_weights"], dtype=np.float32)
    cw = np.asarray(inputs["communication_weights"], dtype=np.float32)
    assert x.shape == (BATCH, D), x.shape

    nc = _get_nc()
    in_maps = _shard_inputs(x, hw, cw)
    res = run_bass_kernel_spmd(nc, in_maps, core_ids=list(range(NCORES)))
    return _gather_output(res.results)
